# revision 9
# baseline (speedup 1.0000x reference)
"""Trainium2 Bass kernel for a 2-layer dense GCN (NodeEncoder).

    out = adj @ relu(adj @ (x@W1) + b1) @ W2 + b2
    N=16384, F_IN=512, HID=1024, OUT=256, adj dense [N, N] fp32.

Key algebraic optimization vs the straightforward order: layer 1 is
reassociated as (adj @ x) @ W1 == adj @ (x @ W1).  Since F_IN=512 <
HID=1024, multiplying adj by the narrower x first costs 275 GF instead
of 550 GF for the big product, and needs NO collective (x is an input,
replicated on every core).  Layer 2 keeps the cheap order h@W2 first.

Total: 438 GF (54.8 GF/core) vs 713 GF for the reference order.

Sharding: adj row-partitioned across 8 NeuronCores (2048 rows/core).
Per core (m = own 2048 adj rows, halves mh of 1024 rows):

  phase 1:  tT   = (adj_c @ x)^T          [512, 2048]   (lhsT = x k-blocks
            stationary, rhs = adjT_c; psum [f, m]; x streamed per half)
  phase 2:  hT   = relu(W1^T @ tT + b1)   [1024, 2048]  (lhsT = W1 blocks,
            rhs = tT tiles; b1 per-partition bias fused into ACT relu)
  phase 3:  s2_c = h_c @ W2               [2048, 256]   (lhsT = hT tiles)
  AG:       s2   = AllGather(s2_c)        [16384, 256]  (per half, 0.5 MiB)
  phase 4:  out2T_c = (adj_c @ s2)^T + b2 [256, 2048]   (lhsT = s2 blocks
            stationary, rhs = adjT_c re-streamed; b2 via ACT Identity)

Matmuls run in bf16 with fp32 PSUM accumulation.
"""

import numpy as np
import ml_dtypes

import concourse.bass as bass
import concourse.mybir as mybir
import concourse.tile as tile
from concourse.bass_utils import run_bass_kernel_spmd
from concourse.tile_sem_assignment import N_PROCS
from concourse.vector_clock import ScopedClock, VectorClock

# ---------------------------------------------------------------------------
# Workaround: the walrus build in this container caps the number of sync-wait
# commands on a Drain instruction; Tile's kernel-tail drain aggregates one
# wait per logical processor and exceeds it.  Split the tail drain into a
# chain of single-wait drains on the same (SP) queue — semantically identical.
# ---------------------------------------------------------------------------


def _drain_and_barrier_split(self, tick_clock, wait_clock):
    gc = tick_clock.global_clock
    for p in range(N_PROCS):
        partial = VectorClock([gc[q] if q == p else 0 for q in range(N_PROCS)])
        d = self.nc.sync.drain()
        wait_clock.add_sem_waits(d.ins, ScopedClock({None: partial}))
    self.nc.sync.drain()

    self.nc.all_engine_barrier()
    assert self.sems is not None
    popped = self.nc._tile_sem_poison_stack.pop()
    assert popped is self._sem_poison
    self.nc.clear_and_free_semaphores(list(self.sems.allocated().values()))
    self.nc.all_engine_barrier()


tile.TileContext._drain_and_barrier = _drain_and_barrier_split

# The same walrus cap applies to every instruction kind: at most ONE sync
# wait command per instruction.  Post-pass: hoist excess sem-waits onto
# no-ops inserted just before the instruction on the same engine queue —
# per-engine program order makes this semantically identical.
_MAX_WAITS = 1


def _split_excess_waits(nc):
    ctr = 0
    for f in nc.m.functions:
        for bb in f.blocks:
            out = []
            changed = False
            for inst in bb.instructions:
                si = inst.sync_info
                waits = list(si.on_wait) if si is not None and si.on_wait else []
                if len(waits) > _MAX_WAITS:
                    changed = True
                    keep, excess = waits[: _MAX_WAITS], waits[_MAX_WAITS :]
                    for i in range(0, len(excess), _MAX_WAITS):
                        ctr += 1
                        nop = mybir.InstNoOp(name=f"I-waitnop-{ctr}")
                        nop.engine = inst.engine
                        nop.sync_info = mybir.SyncInfo(
                            on_wait=excess[i : i + _MAX_WAITS], on_update=[]
                        )
                        out.append(nop)
                    si.on_wait = keep
                out.append(inst)
            if changed:
                bb.instructions = out
    return ctr


def _elide_redundant_ldweights(nc):
    """Delete an InstLdweights that reloads the exact weights AP loaded by
    the previous (surviving) InstLdweights when only plain matmuls / no-ops
    sit between them in the scheduled stream.  The PE array keeps the
    stationary operand across matmuls, so the reload is pure overhead
    (walrus emits one LDWEIGHTS per MATMUL and its ldw-opt pass is
    incompatible with pre-split LDW+MM).  Only sync-free LDWs are removed,
    so semaphore bookkeeping is unchanged."""
    n_elided = 0
    for f in nc.m.functions:
        for bb in f.blocks:
            out = []
            last_w = None  # weights-AP repr of last surviving LDW, if run intact
            changed = False
            for inst in bb.instructions:
                nm = type(inst).__name__
                if nm == "InstLdweights":
                    si = inst.sync_info
                    clean = not (si and (si.on_wait or si.on_update))
                    w = repr(inst.ins[0])
                    if clean and last_w == w:
                        n_elided += 1
                        changed = True
                        continue  # drop the reload
                    last_w = w if clean else None
                elif nm == "InstMatmult":
                    if getattr(inst, "is_transpose", False):
                        last_w = None
                elif nm == "InstNoOp":
                    pass
                else:
                    last_w = None
                out.append(inst)
            if changed:
                bb.instructions = out
    return n_elided


NCORES = 8
N = 16384
SH = N // NCORES  # 2048 adj rows per core
F = 512
HID = 1024
OUT = 256

BF16 = mybir.dt.bfloat16
F32 = mybir.dt.float32

_built = None


def build():
    """Build the per-core Bass program (identical on all cores)."""
    nc = bass.Bass()

    adjT = nc.declare_dram_parameter("adjT", [N, SH], BF16, isOutput=False)
    x = nc.declare_dram_parameter("x", [N, F], BF16, isOutput=False)
    w1 = nc.declare_dram_parameter("w1", [F, HID], BF16, isOutput=False)
    w2 = nc.declare_dram_parameter("w2", [HID, OUT], BF16, isOutput=False)
    b1T = nc.declare_dram_parameter("b1T", [128, HID // 128], F32, isOutput=False)
    b2T = nc.declare_dram_parameter("b2T", [128, OUT // 128], F32, isOutput=False)
    out2T = nc.declare_dram_parameter("out2T", [OUT, SH], F32, isOutput=True)

    rg = [list(range(NCORES))]

    def allgather(inp, outp):
        return nc.gpsimd.collective_compute(
            "AllGather",
            mybir.AluOpType.bypass,
            replica_groups=rg,
            ins=[inp.opt()],
            outs=[outp.opt()],
        )

    with tile.TileContext(nc) as tc:
        with (
            tc.tile_pool(name="const", bufs=1) as constp,
            tc.tile_pool(name="psum", bufs=8, space="PSUM") as psum,
            tc.tile_pool(name="dram", bufs=1, space="DRAM") as dram,
        ):
            # ---- constants (gpsimd queue: idle until the first AllGather,
            # so these don't delay the first x/adjT tiles) ----
            w1t = constp.tile([128, F // 128, HID], BF16)
            nc.gpsimd.dma_start(w1t[:], w1[:].rearrange("(f p) n -> p f n", p=128))
            w2t = constp.tile([128, HID // 128, OUT], BF16)
            nc.gpsimd.dma_start(w2t[:], w2[:].rearrange("(f p) n -> p f n", p=128))
            b1t = constp.tile([128, HID // 128], F32)
            nc.gpsimd.dma_start(b1t[:], b1T[:])
            b2t = constp.tile([128, OUT // 128], F32)
            nc.gpsimd.dma_start(b2t[:], b2T[:])

            # AllGather of s2 split in halves so the second gather overlaps
            # phase-4 compute on the first half's k-blocks.
            ag_in = [dram.tile([SH // 2, OUT], BF16, name=f"agi{h}") for h in range(2)]
            ag_out = [
                dram.tile([N // 2, OUT], BF16, addr_space="Shared", name=f"ago{h}")
                for h in range(2)
            ]

            with (
                # Deep prefetch (20 k-blocks ≈ 49 us of phase-1 compute):
                # while an AllGather occupies the DMA channels (~41 us), the
                # PE keeps consuming already-buffered tiles instead of
                # starving like it would with shallow double-buffering.
                tc.tile_pool(name="adj1", bufs=20) as adjp,
                tc.tile_pool(name="xs", bufs=20) as xp,
                tc.tile_pool(name="tt", bufs=8) as ttp,
                tc.tile_pool(name="ht", bufs=16) as htp,
                tc.tile_pool(name="small", bufs=4) as smallp,
            ):
              for mh in range(2):
                # ---- phase 1: tT[:, mh] = (adj_c[mh rows] @ x)^T ----
                # lhsT = x k-block (stationary, reused for 2 m-slices),
                # rhs = adjT_c k-block; psum[f, m] accumulates over all k.
                ps1 = [
                    psum.tile([128, 512], F32, tag="ps", name=f"ps1_{mh}_{i}")
                    for i in range(8)
                ]
                for kb in range(128):
                    xt = xp.tile([128, F], BF16, tag="xt", name=f"xt{mh}_{kb}")
                    nc.scalar.dma_start(xt[:], x[kb * 128 : (kb + 1) * 128, :])
                    at = adjp.tile([128, 1024], BF16, tag="at", name=f"at{mh}_{kb}")
                    nc.sync.dma_start(
                        at[:],
                        adjT[kb * 128 : (kb + 1) * 128, mh * 1024 : (mh + 1) * 1024],
                    )
                    for fb in range(4):
                        for msl in range(2):
                            nc.tensor.matmul(
                                ps1[fb * 2 + msl][:],
                                xt[:, fb * 128 : (fb + 1) * 128],
                                at[:, msl * 512 : (msl + 1) * 512],
                                start=(kb == 0),
                                stop=(kb == 127),
                            )
                tt = {}
                for fb in range(4):
                    for msl in range(2):
                        t = ttp.tile([128, 512], BF16, tag="tt", name=f"tt{mh}_{fb}_{msl}")
                        nc.vector.tensor_copy(t[:], ps1[fb * 2 + msl][:])
                        tt[(fb, msl)] = t

                # ---- phase 2: hT[:, mh] = relu(W1^T @ tT + b1) ----
                # lhsT = W1 [f,n]-block (stationary, reused for 2 m-slices).
                ht = {}
                for nbh in range(2):
                    ps2 = [
                        psum.tile([128, 512], F32, tag="ps", name=f"ps2_{mh}_{nbh}_{i}")
                        for i in range(8)
                    ]
                    for fb in range(4):
                        for nb in range(4):
                            j = nbh * 4 + nb
                            for msl in range(2):
                                nc.tensor.matmul(
                                    ps2[nb * 2 + msl][:],
                                    w1t[:, fb, j * 128 : (j + 1) * 128],
                                    tt[(fb, msl)][:],
                                    start=(fb == 0),
                                    stop=(fb == 3),
                                )
                    for nb in range(4):
                        j = nbh * 4 + nb
                        for msl in range(2):
                            htt = htp.tile(
                                [128, 512], BF16, tag="ht", name=f"ht{mh}_{j}_{msl}"
                            )
                            nc.scalar.activation(
                                htt[:],
                                ps2[nb * 2 + msl][:],
                                mybir.ActivationFunctionType.Relu,
                                bias=b1t[:, j : j + 1],
                            )
                            ht[(j, msl)] = htt

                # ---- phase 3: s2_c[mh] = h_c[mh] @ W2, then gather ----
                for mb in range(8):
                    ps3 = psum.tile([128, OUT], F32, tag="ps", name=f"ps3_{mh}_{mb}")
                    for j in range(8):
                        nc.tensor.matmul(
                            ps3[:],
                            ht[(j, mb // 4)][:, (mb % 4) * 128 : (mb % 4 + 1) * 128],
                            w2t[:, j, :],
                            start=(j == 0),
                            stop=(j == 7),
                        )
                    s2o = smallp.tile([128, OUT], BF16, tag="s2o", name=f"s2o{mh}_{mb}")
                    nc.vector.tensor_copy(s2o[:], ps3[:])
                    nc.scalar.dma_start(ag_in[mh][mb * 128 : (mb + 1) * 128, :], s2o[:])
                allgather(ag_in[mh], ag_out[mh])

            # ---- phase 4: out2T = (adj_c @ s2)^T + b2 ----
            # All 8 psum banks accumulate concurrently over 128 k-blocks;
            # k-blocks consumed in gather-half order so half-1 compute
            # overlaps the second AllGather.  lhsT = s2 [k,o]-block
            # (stationary, reused for 4 m-slices), rhs = adjT_c re-streamed.
            with (
                tc.tile_pool(name="adj4", bufs=20) as adj4p,
                tc.tile_pool(name="s2l", bufs=2) as s2p,
                tc.tile_pool(name="outp", bufs=8) as outp,
            ):
                # ag_out[h] rows = g*1024 + kbl*128 + p  (rank g, half h)
                s2_srcs = [
                    ag_out[h][:].rearrange("(g kb p) o -> p g kb o", g=8, p=128)
                    for h in range(2)
                ]
                ps4 = [
                    psum.tile([128, 512], F32, tag="ps", name=f"ps4_{i}")
                    for i in range(8)
                ]
                ki = 0
                for mh in range(2):
                    for g in range(8):
                        st = s2p.tile([128, 8, OUT], BF16, tag="st", name=f"st{mh}_{g}")
                        nc.scalar.dma_start(st[:], s2_srcs[mh][:, g])
                        for kbl in range(8):
                            kb = g * 16 + mh * 8 + kbl
                            at = adj4p.tile([128, SH], BF16, tag="at4", name=f"at4_{kb}")
                            nc.sync.dma_start(
                                at[:], adjT[kb * 128 : (kb + 1) * 128, :]
                            )
                            for ob in range(2):
                                for msl in range(4):
                                    nc.tensor.matmul(
                                        ps4[ob * 4 + msl][:],
                                        st[:, kbl, ob * 128 : (ob + 1) * 128],
                                        at[:, msl * 512 : (msl + 1) * 512],
                                        start=(ki == 0),
                                        stop=(ki == 127),
                                    )
                            ki += 1
                # tail drain: the whole 2 MiB writeback is serial latency at
                # kernel end, so split the bias-add across scalar ACT and
                # vector (per-partition tensor_scalar_add) and the stores
                # across two DMA queues.
                for ob in range(2):
                    for msl in range(4):
                        i = ob * 4 + msl
                        ot = outp.tile([128, 512], F32, tag="ot", name=f"ot{ob}_{msl}")
                        if i % 2 == 0:
                            nc.scalar.activation(
                                ot[:],
                                ps4[i][:],
                                mybir.ActivationFunctionType.Identity,
                                bias=b2t[:, ob : ob + 1],
                            )
                        else:
                            nc.vector.tensor_scalar_add(
                                ot[:], ps4[i][:], b2t[:, ob : ob + 1]
                            )
                        q = nc.sync if i % 2 == 0 else nc.scalar
                        q.dma_start(
                            out2T[
                                ob * 128 : (ob + 1) * 128, msl * 512 : (msl + 1) * 512
                            ],
                            ot[:],
                        )

    _elide_redundant_ldweights(nc)
    _split_excess_waits(nc)
    return nc


def _prep_inputs(x, adj, W1, b1, W2, b2):
    bf = ml_dtypes.bfloat16
    xb = x.astype(bf)
    w1b = W1.astype(bf)
    w2b = W2.astype(bf)
    b1T = np.ascontiguousarray(b1.reshape(HID // 128, 128).T).astype(np.float32)
    b2T = np.ascontiguousarray(b2.reshape(OUT // 128, 128).T).astype(np.float32)
    in_maps = []
    for c in range(NCORES):
        rows = slice(c * SH, (c + 1) * SH)
        in_maps.append(
            {
                "adjT": np.ascontiguousarray(adj[rows, :].T).astype(bf),
                "x": xb,
                "w1": w1b,
                "w2": w2b,
                "b1T": b1T,
                "b2T": b2T,
            }
        )
    return in_maps


def _run(inputs, trace=False):
    global _built
    if _built is None:
        _built = build()
    in_maps = _prep_inputs(**inputs)
    r = run_bass_kernel_spmd(_built, in_maps, list(range(NCORES)), trace=trace)
    out = np.empty([N, OUT], np.float32)
    for c in range(NCORES):
        out[c * SH : (c + 1) * SH, :] = r.results[c]["out2T"].T
    return out, r


def kernel(x, adj, W1, b1, W2, b2):
    out, _ = _run(dict(x=x, adj=adj, W1=W1, b1=b1, W2=W2, b2=b2))
    return out


# revision 10
# speedup vs baseline: 1.0023x; 1.0023x over previous
"""Trainium2 Bass kernel for a 2-layer dense GCN (NodeEncoder).

    out = adj @ relu(adj @ (x@W1) + b1) @ W2 + b2
    N=16384, F_IN=512, HID=1024, OUT=256, adj dense [N, N] fp32.

Key algebraic optimization vs the straightforward order: layer 1 is
reassociated as (adj @ x) @ W1 == adj @ (x @ W1).  Since F_IN=512 <
HID=1024, multiplying adj by the narrower x first costs 275 GF instead
of 550 GF for the big product, and needs NO collective (x is an input,
replicated on every core).  Layer 2 keeps the cheap order h@W2 first.

Total: 438 GF (54.8 GF/core) vs 713 GF for the reference order.

Sharding: adj row-partitioned across 8 NeuronCores (2048 rows/core).
Per core (m = own 2048 adj rows, halves mh of 1024 rows):

  phase 1:  tT   = (adj_c @ x)^T          [512, 2048]   (lhsT = x k-blocks
            stationary, rhs = adjT_c; psum [f, m]; x streamed per half)
  phase 2:  hT   = relu(W1^T @ tT + b1)   [1024, 2048]  (lhsT = W1 blocks,
            rhs = tT tiles; b1 per-partition bias fused into ACT relu)
  phase 3:  s2_c = h_c @ W2               [2048, 256]   (lhsT = hT tiles)
  AG:       s2   = AllGather(s2_c)        [16384, 256]  (per half, 0.5 MiB)
  phase 4:  out2T_c = (adj_c @ s2)^T + b2 [256, 2048]   (lhsT = s2 blocks
            stationary, rhs = adjT_c re-streamed; b2 via ACT Identity)

Matmuls run in bf16 with fp32 PSUM accumulation.
"""

import numpy as np
import ml_dtypes

import concourse.bass as bass
import concourse.mybir as mybir
import concourse.tile as tile
from concourse.bass_utils import run_bass_kernel_spmd
from concourse.tile_sem_assignment import N_PROCS
from concourse.vector_clock import ScopedClock, VectorClock

# ---------------------------------------------------------------------------
# Workaround: the walrus build in this container caps the number of sync-wait
# commands on a Drain instruction; Tile's kernel-tail drain aggregates one
# wait per logical processor and exceeds it.  Split the tail drain into a
# chain of single-wait drains on the same (SP) queue — semantically identical.
# ---------------------------------------------------------------------------


def _drain_and_barrier_split(self, tick_clock, wait_clock):
    gc = tick_clock.global_clock
    for p in range(N_PROCS):
        partial = VectorClock([gc[q] if q == p else 0 for q in range(N_PROCS)])
        d = self.nc.sync.drain()
        wait_clock.add_sem_waits(d.ins, ScopedClock({None: partial}))
    self.nc.sync.drain()

    self.nc.all_engine_barrier()
    assert self.sems is not None
    popped = self.nc._tile_sem_poison_stack.pop()
    assert popped is self._sem_poison
    self.nc.clear_and_free_semaphores(list(self.sems.allocated().values()))
    self.nc.all_engine_barrier()


tile.TileContext._drain_and_barrier = _drain_and_barrier_split

# The same walrus cap applies to every instruction kind: at most ONE sync
# wait command per instruction.  Post-pass: hoist excess sem-waits onto
# no-ops inserted just before the instruction on the same engine queue —
# per-engine program order makes this semantically identical.
_MAX_WAITS = 1


def _split_excess_waits(nc):
    ctr = 0
    for f in nc.m.functions:
        for bb in f.blocks:
            out = []
            changed = False
            for inst in bb.instructions:
                si = inst.sync_info
                waits = list(si.on_wait) if si is not None and si.on_wait else []
                if len(waits) > _MAX_WAITS:
                    changed = True
                    keep, excess = waits[: _MAX_WAITS], waits[_MAX_WAITS :]
                    for i in range(0, len(excess), _MAX_WAITS):
                        ctr += 1
                        nop = mybir.InstNoOp(name=f"I-waitnop-{ctr}")
                        nop.engine = inst.engine
                        nop.sync_info = mybir.SyncInfo(
                            on_wait=excess[i : i + _MAX_WAITS], on_update=[]
                        )
                        out.append(nop)
                    si.on_wait = keep
                out.append(inst)
            if changed:
                bb.instructions = out
    return ctr


def _elide_redundant_ldweights(nc):
    """Delete an InstLdweights that reloads the exact weights AP loaded by
    the previous (surviving) InstLdweights when only plain matmuls / no-ops
    sit between them in the scheduled stream.  The PE array keeps the
    stationary operand across matmuls, so the reload is pure overhead
    (walrus emits one LDWEIGHTS per MATMUL and its ldw-opt pass is
    incompatible with pre-split LDW+MM).  Only sync-free LDWs are removed,
    so semaphore bookkeeping is unchanged."""
    n_elided = 0
    for f in nc.m.functions:
        for bb in f.blocks:
            out = []
            last_w = None  # weights-AP repr of last surviving LDW, if run intact
            changed = False
            for inst in bb.instructions:
                nm = type(inst).__name__
                if nm == "InstLdweights":
                    si = inst.sync_info
                    clean = not (si and (si.on_wait or si.on_update))
                    w = repr(inst.ins[0])
                    if clean and last_w == w:
                        n_elided += 1
                        changed = True
                        continue  # drop the reload
                    last_w = w if clean else None
                elif nm == "InstMatmult":
                    if getattr(inst, "is_transpose", False):
                        last_w = None
                elif nm == "InstNoOp":
                    pass
                else:
                    last_w = None
                out.append(inst)
            if changed:
                bb.instructions = out
    return n_elided


NCORES = 8
N = 16384
SH = N // NCORES  # 2048 adj rows per core
F = 512
HID = 1024
OUT = 256

BF16 = mybir.dt.bfloat16
F32 = mybir.dt.float32

_built = None


def build():
    """Build the per-core Bass program (identical on all cores)."""
    nc = bass.Bass()

    adjT = nc.declare_dram_parameter("adjT", [N, SH], BF16, isOutput=False)
    x = nc.declare_dram_parameter("x", [N, F], BF16, isOutput=False)
    w1 = nc.declare_dram_parameter("w1", [F, HID], BF16, isOutput=False)
    w2 = nc.declare_dram_parameter("w2", [HID, OUT], BF16, isOutput=False)
    b1T = nc.declare_dram_parameter("b1T", [128, HID // 128], F32, isOutput=False)
    b2T = nc.declare_dram_parameter("b2T", [128, OUT // 128], F32, isOutput=False)
    out2T = nc.declare_dram_parameter("out2T", [OUT, SH], F32, isOutput=True)

    rg = [list(range(NCORES))]

    def allgather(inp, outp):
        return nc.gpsimd.collective_compute(
            "AllGather",
            mybir.AluOpType.bypass,
            replica_groups=rg,
            ins=[inp.opt()],
            outs=[outp.opt()],
        )

    with tile.TileContext(nc) as tc:
        with (
            tc.tile_pool(name="const", bufs=1) as constp,
            tc.tile_pool(name="psum", bufs=8, space="PSUM") as psum,
            tc.tile_pool(name="dram", bufs=1, space="DRAM") as dram,
        ):
            # ---- constants (gpsimd queue: idle until the first AllGather,
            # so these don't delay the first x/adjT tiles) ----
            w1t = constp.tile([128, F // 128, HID], BF16)
            nc.gpsimd.dma_start(w1t[:], w1[:].rearrange("(f p) n -> p f n", p=128))
            w2t = constp.tile([128, HID // 128, OUT], BF16)
            nc.gpsimd.dma_start(w2t[:], w2[:].rearrange("(f p) n -> p f n", p=128))
            b1t = constp.tile([128, HID // 128], F32)
            nc.gpsimd.dma_start(b1t[:], b1T[:])
            b2t = constp.tile([128, OUT // 128], F32)
            nc.gpsimd.dma_start(b2t[:], b2T[:])

            # AllGather of s2 split in halves so the second gather overlaps
            # phase-4 compute on the first half's k-blocks.
            ag_in = [dram.tile([SH // 2, OUT], BF16, name=f"agi{h}") for h in range(2)]
            ag_out = [
                dram.tile([N // 2, OUT], BF16, addr_space="Shared", name=f"ago{h}")
                for h in range(2)
            ]

            with (
                # Deep prefetch (20 k-blocks ≈ 49 us of phase-1 compute):
                # while an AllGather occupies the DMA channels (~41 us), the
                # PE keeps consuming already-buffered tiles instead of
                # starving like it would with shallow double-buffering.
                tc.tile_pool(name="adj1", bufs=20) as adjp,
                tc.tile_pool(name="xs", bufs=20) as xp,
                tc.tile_pool(name="tt", bufs=8) as ttp,
                tc.tile_pool(name="ht", bufs=16) as htp,
                tc.tile_pool(name="small", bufs=4) as smallp,
            ):
              for mh in range(2):
                # ---- phase 1: tT[:, mh] = (adj_c[mh rows] @ x)^T ----
                # lhsT = x k-block (stationary, reused for 2 m-slices),
                # rhs = adjT_c k-block; psum[f, m] accumulates over all k.
                ps1 = [
                    psum.tile([128, 512], F32, tag="ps", name=f"ps1_{mh}_{i}")
                    for i in range(8)
                ]
                for kb in range(128):
                    xt = xp.tile([128, F], BF16, tag="xt", name=f"xt{mh}_{kb}")
                    nc.scalar.dma_start(xt[:], x[kb * 128 : (kb + 1) * 128, :])
                    at = adjp.tile([128, 1024], BF16, tag="at", name=f"at{mh}_{kb}")
                    nc.sync.dma_start(
                        at[:],
                        adjT[kb * 128 : (kb + 1) * 128, mh * 1024 : (mh + 1) * 1024],
                    )
                    for fb in range(4):
                        for msl in range(2):
                            nc.tensor.matmul(
                                ps1[fb * 2 + msl][:],
                                xt[:, fb * 128 : (fb + 1) * 128],
                                at[:, msl * 512 : (msl + 1) * 512],
                                start=(kb == 0),
                                stop=(kb == 127),
                            )
                tt = {}
                for fb in range(4):
                    for msl in range(2):
                        t = ttp.tile([128, 512], BF16, tag="tt", name=f"tt{mh}_{fb}_{msl}")
                        nc.vector.tensor_copy(t[:], ps1[fb * 2 + msl][:])
                        tt[(fb, msl)] = t

                # ---- phase 2: hT[:, mh] = relu(W1^T @ tT + b1) ----
                # lhsT = W1 [f,n]-block (stationary, reused for 2 m-slices).
                ht = {}
                for nbh in range(2):
                    ps2 = [
                        psum.tile([128, 512], F32, tag="ps", name=f"ps2_{mh}_{nbh}_{i}")
                        for i in range(8)
                    ]
                    for fb in range(4):
                        for nb in range(4):
                            j = nbh * 4 + nb
                            for msl in range(2):
                                nc.tensor.matmul(
                                    ps2[nb * 2 + msl][:],
                                    w1t[:, fb, j * 128 : (j + 1) * 128],
                                    tt[(fb, msl)][:],
                                    start=(fb == 0),
                                    stop=(fb == 3),
                                )
                    for nb in range(4):
                        j = nbh * 4 + nb
                        for msl in range(2):
                            htt = htp.tile(
                                [128, 512], BF16, tag="ht", name=f"ht{mh}_{j}_{msl}"
                            )
                            nc.scalar.activation(
                                htt[:],
                                ps2[nb * 2 + msl][:],
                                mybir.ActivationFunctionType.Relu,
                                bias=b1t[:, j : j + 1],
                            )
                            ht[(j, msl)] = htt

                # ---- phase 3: s2_c[mh] = h_c[mh] @ W2, then gather ----
                for mb in range(8):
                    ps3 = psum.tile([128, OUT], F32, tag="ps", name=f"ps3_{mh}_{mb}")
                    for j in range(8):
                        nc.tensor.matmul(
                            ps3[:],
                            ht[(j, mb // 4)][:, (mb % 4) * 128 : (mb % 4 + 1) * 128],
                            w2t[:, j, :],
                            start=(j == 0),
                            stop=(j == 7),
                        )
                    s2o = smallp.tile([128, OUT], BF16, tag="s2o", name=f"s2o{mh}_{mb}")
                    nc.vector.tensor_copy(s2o[:], ps3[:])
                    nc.scalar.dma_start(ag_in[mh][mb * 128 : (mb + 1) * 128, :], s2o[:])
                allgather(ag_in[mh], ag_out[mh])

            # ---- phase 4: out2T = (adj_c @ s2)^T + b2 ----
            # All 8 psum banks accumulate concurrently over 128 k-blocks;
            # k-blocks consumed in gather-half order so half-1 compute
            # overlaps the second AllGather.  lhsT = s2 [k,o]-block
            # (stationary, reused for 4 m-slices), rhs = adjT_c re-streamed.
            with (
                tc.tile_pool(name="adj4", bufs=20) as adj4p,
                tc.tile_pool(name="s2l", bufs=24) as s2p,
                tc.tile_pool(name="outp", bufs=8) as outp,
            ):
                # ag_out[h] rows = g*1024 + kbl*128 + p  (rank g, half h)
                s2_srcs = [
                    ag_out[h][:].rearrange("(g kb p) o -> p g kb o", g=8, p=128)
                    for h in range(2)
                ]
                ps4 = [
                    psum.tile([128, 512], F32, tag="ps", name=f"ps4_{i}")
                    for i in range(8)
                ]
                # s2 stationary tiles loaded as small [128, OUT] patches, one
                # g-group ahead of consumption: the first patch lands right
                # away instead of queueing behind the 10 MiB adjT prefetch
                # burst, and the mh=1 group's triggers (which must wait on
                # the second AllGather) never block anything critical.
                st = {}

                def load_st(mh, g):
                    tiles = []
                    for kbl in range(8):
                        t = s2p.tile([128, OUT], BF16, tag="st", name=f"st{mh}_{g}_{kbl}")
                        nc.scalar.dma_start(t[:], s2_srcs[mh][:, g, kbl])
                        tiles.append(t)
                    st[(mh, g)] = tiles

                load_st(0, 0)
                ki = 0
                for mh in range(2):
                    for g in range(8):
                        nxt = (mh, g + 1) if g < 7 else (mh + 1, 0)
                        if nxt[0] < 2 and nxt not in st:
                            load_st(*nxt)
                        for kbl in range(8):
                            kb = g * 16 + mh * 8 + kbl
                            at = adj4p.tile([128, SH], BF16, tag="at4", name=f"at4_{kb}")
                            nc.sync.dma_start(
                                at[:], adjT[kb * 128 : (kb + 1) * 128, :]
                            )
                            for ob in range(2):
                                for msl in range(4):
                                    nc.tensor.matmul(
                                        ps4[ob * 4 + msl][:],
                                        st[(mh, g)][kbl][:, ob * 128 : (ob + 1) * 128],
                                        at[:, msl * 512 : (msl + 1) * 512],
                                        start=(ki == 0),
                                        stop=(ki == 127),
                                    )
                            ki += 1
                # tail drain: the whole 2 MiB writeback is serial latency at
                # kernel end, so split the bias-add across scalar ACT and
                # vector (per-partition tensor_scalar_add) and the stores
                # across two DMA queues.
                for ob in range(2):
                    for msl in range(4):
                        i = ob * 4 + msl
                        ot = outp.tile([128, 512], F32, tag="ot", name=f"ot{ob}_{msl}")
                        if i % 2 == 0:
                            nc.scalar.activation(
                                ot[:],
                                ps4[i][:],
                                mybir.ActivationFunctionType.Identity,
                                bias=b2t[:, ob : ob + 1],
                            )
                        else:
                            nc.vector.tensor_scalar_add(
                                ot[:], ps4[i][:], b2t[:, ob : ob + 1]
                            )
                        q = nc.sync if i % 2 == 0 else nc.scalar
                        q.dma_start(
                            out2T[
                                ob * 128 : (ob + 1) * 128, msl * 512 : (msl + 1) * 512
                            ],
                            ot[:],
                        )

    _elide_redundant_ldweights(nc)
    _split_excess_waits(nc)
    return nc


def _prep_inputs(x, adj, W1, b1, W2, b2):
    bf = ml_dtypes.bfloat16
    xb = x.astype(bf)
    w1b = W1.astype(bf)
    w2b = W2.astype(bf)
    b1T = np.ascontiguousarray(b1.reshape(HID // 128, 128).T).astype(np.float32)
    b2T = np.ascontiguousarray(b2.reshape(OUT // 128, 128).T).astype(np.float32)
    in_maps = []
    for c in range(NCORES):
        rows = slice(c * SH, (c + 1) * SH)
        in_maps.append(
            {
                "adjT": np.ascontiguousarray(adj[rows, :].T).astype(bf),
                "x": xb,
                "w1": w1b,
                "w2": w2b,
                "b1T": b1T,
                "b2T": b2T,
            }
        )
    return in_maps


def _run(inputs, trace=False):
    global _built
    if _built is None:
        _built = build()
    in_maps = _prep_inputs(**inputs)
    r = run_bass_kernel_spmd(_built, in_maps, list(range(NCORES)), trace=trace)
    out = np.empty([N, OUT], np.float32)
    for c in range(NCORES):
        out[c * SH : (c + 1) * SH, :] = r.results[c]["out2T"].T
    return out, r


def kernel(x, adj, W1, b1, W2, b2):
    out, _ = _run(dict(x=x, adj=adj, W1=W1, b1=b1, W2=W2, b2=b2))
    return out
